# revision 64
# baseline (speedup 1.0000x reference)
"""GAT layer (dense formulation) on 8 Trainium2 NeuronCores.

Computation (N=4096 nodes, IN_F=512, OUT_F=64, HEADS=4):
    h = (x @ W).reshape(N, H, F)
    s = h . a_src ; t = h . a_dst            (per node, per head)
    e[i,j,k] = leaky_relu(s[i,k] + t[j,k])   masked by adj[i,j]
    attn = softmax_j(e) ; out = attn @ h

Sharding: output rows i (nodes) are sharded 512/core across 8 cores.
Each core computes the full h = x @ W redundantly (cheap), then handles
its own 512 i-rows: logits laid out [j=partitions, i=free] so that the
softmax contraction over j runs on the PE as  [h_k | 1].T @ at_tile,
with the ones-column producing the softmax denominator for free.

Factorized attention — no per-element exp at all. Using
    lrelu(v) = 0.2 v + 0.8 relu(v),   v = s_i + t_j:
    exp(lrelu(v)) = e^{0.2 s} * e^{0.2 t} * max(e^{0.8 v}, 1)
The e^{0.2 s_i} factor is constant along the softmax axis j, so it
cancels between numerator and denominator and is simply dropped. With a
global shift e^{-c} (also cancels) the attention surrogate is
    at[j,i] = max( e^{0.8 s_i} * e^{t_j - c},  e^{0.2 t_j - c} ) * m01
= ONE tensor_scalar (mult,max — 4x bf16) + ONE 2048-wide tensor_tensor
mult (2x bf16, mask broadcast across heads) per tile on the DVE; the
per-partition exp scalars come from a single tiny [128, 8] ACT exp per
tile, with the t and 0.2t columns produced by the h-matmul itself via
extra folded W columns. Exact math (no approximation beyond bf16).

Schedule: all DMAs batched into multi-level-AP transfers (successive
dma_start ops serialize ~0.7us each); phase C (s broadcast) per-head
chains interleaved with the first h tiles so the DVE starts ASAP; the
whole h-phase is issued before the elementwise/attention phase so the
in-order PE queue never blocks h-matmuls behind attention matmuls that
wait on the DVE. The DVE is the pacing engine (~2.7us/tile).
"""

import os

import numpy as np
import ml_dtypes

import concourse.bass as bass
import concourse.mybir as mybir
import concourse.tile as tile
from concourse import bacc, bass_utils
from concourse._compat import get_trn_type
from concourse.alu_op_type import AluOpType

# ---------------------------------------------------------------- constants
N = 4096
IN_F = 512
OUT_F = 64
HEADS = 4
ALPHA = 0.2
NCORES = 8
SHARD = N // NCORES            # 512 output rows per core
NT = N // 128                  # 32 j-tiles (and n-tiles)
KC = IN_F // 128               # 4 contraction chunks
# W_ext = [W(256) | Ws(4) | Wt(4) | 0.2*Wt(4)]
WCOLS = HEADS * OUT_F + 3 * HEADS   # 268
TCOL = HEADS * OUT_F + HEADS        # 260: start of the [t | 0.2t] block
HB = OUT_F + 1                 # 65 = per-head [h_k | ones] weight block
CSHIFT = 3.0                   # global exp shift (cancels in softmax)

F32 = mybir.dt.float32
BF16 = mybir.dt.bfloat16

# ------------------------------------------------------------- bass program
_PROGRAM = None


def _build_program():
    """One SPMD program; per-core behavior differs only through input data."""
    global _PROGRAM
    if _PROGRAM is not None:
        return _PROGRAM

    nc = bacc.Bacc(get_trn_type() or "TRN2", target_bir_lowering=False)
    act = mybir.ActivationFunctionType

    # x^T, full: xT[f, n] = x[n, f], bf16
    xt_d = nc.dram_tensor("xT", [IN_F, N], BF16, kind="ExternalInput")
    # x-shard transposed: xsT[f, i] = x[shard_start + i, f], bf16
    xs_d = nc.dram_tensor("xsT", [IN_F, SHARD], BF16, kind="ExternalInput")
    w_d = nc.dram_tensor("wext", [IN_F, WCOLS], BF16, kind="ExternalInput")
    # Ws replicated across 128 columns per head (for one-shot s broadcast)
    wr_d = nc.dram_tensor("wrep", [IN_F, HEADS * 128], BF16, kind="ExternalInput")
    # multiplicative mask {0,1}, pre-tiled on host: m01[b, p, q*i] =
    #   (adj.T[b*512 + q*128 + p, shard_i] != 0)   (bf16; 4KB DMA lines)
    m_d = nc.dram_tensor("maskTb", [NT // 4, 128, 4 * SHARD], BF16,
                         kind="ExternalInput")
    out_d = nc.dram_tensor("out", [SHARD, HEADS * OUT_F], F32, kind="ExternalOutput")

    with tile.TileContext(nc) as tc:
        with (
            tc.tile_pool(name="const", bufs=1) as cp,
            tc.tile_pool(name="hpool", bufs=1) as hp,
            tc.tile_pool(name="mpool", bufs=1) as mp,
            tc.tile_pool(name="work", bufs=6) as wp,
            tc.tile_pool(name="endp", bufs=2) as ep,
            tc.tile_pool(name="ps", bufs=2, space="PSUM") as psp,
            tc.tile_pool(name="ph", bufs=2, space="PSUM") as php,
            tc.tile_pool(name="psacc", bufs=1, space="PSUM") as psa,
        ):
            # ---------------- phase A: constants in (one DMA per tensor —
            # successive dma_start instructions serialize at ~0.7us each on
            # the sync queue, so batch via multi-level access patterns)
            xst_b = cp.tile([128, KC * SHARD], BF16, name="xst", tag="xst")
            nc.sync.dma_start(
                xst_b.rearrange("p (k i) -> p k i", k=KC),
                xs_d.rearrange("(k p) i -> p k i", p=128),
            )
            xst = [xst_b[:, k * SHARD : (k + 1) * SHARD] for k in range(KC)]
            wrep_b = cp.tile([128, KC * HEADS * 128], BF16, name="wrep",
                             tag="wrep")
            nc.sync.dma_start(
                wrep_b.rearrange("p (k c) -> p k c", k=KC),
                wr_d.rearrange("(k p) c -> p k c", p=128),
            )
            wrep = [
                wrep_b[:, k * HEADS * 128 : (k + 1) * HEADS * 128]
                for k in range(KC)
            ]
            wsb_b = cp.tile([128, KC * WCOLS], BF16, name="wsb", tag="wsb")
            nc.sync.dma_start(
                wsb_b.rearrange("p (k c) -> p k c", k=KC),
                w_d.rearrange("(k p) c -> p k c", p=128),
            )
            wsb = [wsb_b[:, k * WCOLS : (k + 1) * WCOLS] for k in range(KC)]
            # full x^T: [KC][4 column groups] tiles of [128, 1024] (2KB lines).
            # Group-wise loads let the h-phase start after the first group.
            msb = [
                mp.tile([128, 4 * SHARD], BF16, name=f"msb{b}", tag=f"msb{b}")
                for b in range(NT // 4)
            ]

            def load_mask(b):
                nc.sync.dma_start(msb[b], m_d[b])

            GB = [0, 1024, 2048, 3072, 4096]
            NG = len(GB) - 1
            xsb_g = []
            for g in range(NG):
                gw = GB[g + 1] - GB[g]
                x_t = cp.tile([128, KC * gw], BF16, name=f"xsb_{g}",
                              tag=f"xsb_{g}")
                nc.sync.dma_start(
                    x_t.rearrange("p (k n) -> p k n", k=KC),
                    xt_d[:, GB[g] : GB[g + 1]].rearrange(
                        "(k p) n -> p k n", p=128
                    ),
                )
                xsb_g.append(x_t)
                if g < 2:
                    load_mask(g)
            ident = cp.tile([128, 128], F32, name="ident", tag="ident")
            from concourse.masks import make_identity

            make_identity(nc, ident)
            # warm the ACT exp table while DMAs land
            exp_warm = cp.tile([1, 128], F32, name="exp_warm", tag="exp_warm")
            nc.scalar.activation(exp_warm, ident[0:1, :], act.Exp)
            # keep the PE busy during the DMA wait so its clock is ramped
            # when the first real (es8/h) matmuls arrive
            for _ in range(12):
                wps = psp.tile([128, 128], F32, name="warm", tag="pstmp")
                nc.tensor.matmul(wps, lhsT=ident, rhs=ident, start=True,
                                 stop=True)
            ones_row_f32 = cp.tile([1, 128], F32, name="ones_row_f32",
                                   tag="ones_row_f32")
            nc.gpsimd.memset(ones_row_f32, 1.0)
            ones_col_f32 = cp.tile([128, HEADS], F32, name="ones_col_f32",
                                   tag="ones_col_f32")
            nc.gpsimd.memset(ones_col_f32, 1.0)
            cbias = cp.tile([128, 1], F32, name="cbias", tag="cbias")
            nc.gpsimd.memset(cbias, -CSHIFT)

            # ---------------- phases C+B interleaved.
            # C: es8_b4[j, k*512+i] = exp(0.8 * s[shard_i, k]) broadcast.
            # B: all h-compute. The PE queue runs the h matmuls gated only on
            # DMA; the phase-C broadcast matmuls are interleaved between the
            # first h tiles so PE never stalls on phase-C's ACT copies. The
            # DVE elementwise (phase D) starts as soon as es8_b4 lands.
            acc = [
                psa.tile([HB, SHARD], F32, name=f"acc{k}", tag=f"acc{k}")
                for k in range(HEADS)
            ]
            es8_b4 = cp.tile([128, HEADS * SHARD], BF16, name="es8_b4", tag="es8_b4")

            def rep_es8(k):
                # s broadcast in ONE matmul per head: the stationary is Ws_k
                # replicated across all 128 columns (host-prepared), so every
                # output partition gets the same s row — no copy, no second
                # broadcast matmul.
                sb_ps = psp.tile([128, SHARD], F32, name="sb_ps", tag="pstmp")
                for kc in range(KC):
                    nc.tensor.matmul(
                        sb_ps,
                        lhsT=wrep[kc][:, k * 128 : (k + 1) * 128],
                        rhs=xst[kc],
                        start=(kc == 0),
                        stop=(kc == KC - 1),
                    )
                nc.scalar.activation(
                    es8_b4[:, k * SHARD : (k + 1) * SHARD], sb_ps, act.Exp,
                    scale=1.0 - ALPHA,
                )

            h_sb = []
            et82_sb = []

            def h_tile(nt):
                if nt % 4 == 0 and 2 + nt // 4 < NT // 4:
                    load_mask(2 + nt // 4)
                ph = php.tile([128, WCOLS], F32, name="ph", tag="ph")
                col = nt * 128
                g = max(i for i in range(NG) if GB[i] <= col)
                gw, r = GB[g + 1] - GB[g], (col - GB[g]) // 128
                for k in range(KC):
                    nc.tensor.matmul(
                        ph,
                        lhsT=xsb_g[g][:, k * gw + r * 128 : k * gw + (r + 1) * 128],
                        rhs=wsb[k],
                        start=(k == 0),
                        stop=(k == KC - 1),
                    )
                # per-partition exp scalars: [e^{t-c} (4) | e^{0.2t-c} (4)]
                et82 = hp.tile([128, 2 * HEADS], F32, name=f"et82_{nt}",
                               tag=f"et82_{nt}")
                nc.scalar.activation(et82, ph[:, TCOL:], act.Exp, bias=cbias)
                # pack h into weights layout [h0|1|h1|1|h2|1|h3|1]
                h_t = hp.tile([128, HEADS * HB], BF16,
                              name=f"h_sb{nt}", tag=f"h_sb{nt}")
                nc.gpsimd.tensor_copy(
                    h_t.rearrange("p (h c) -> p h c", c=HB)[
                        :, :, OUT_F : OUT_F + 1
                    ],
                    ones_col_f32.rearrange("p (h c) -> p h c", c=1),
                )
                nc.scalar.copy(
                    h_t.rearrange("p (h c) -> p h c", c=HB)[:, :, :OUT_F],
                    ph[:, : HEADS * OUT_F].rearrange("p (h c) -> p h c", c=OUT_F),
                )
                h_sb.append(h_t)
                et82_sb.append(et82)

            # head-0's s chain first so the DVE can start ASAP, then the other
            # heads' chains interleaved between early h tiles.
            rep_es8(0)
            h_tile(0)
            rep_es8(1)
            h_tile(1)
            rep_es8(2)
            h_tile(2)
            rep_es8(3)
            h_tile(3)
            for nt in range(HEADS, NT):
                h_tile(nt)

            # ---------------- phase D: elementwise surrogate + attn matmuls
            #   at_k = max(es8 * e^{t_k - c}, e^{0.2 t_k - c})   (TS, 4x)
            #   at   = at * m01  (one 2048-wide TT, mask broadcast over heads)
            for jt in range(NT):
                et82 = et82_sb[jt]
                at = wp.tile([128, HEADS * SHARD], BF16, name="at", tag="at")
                for k in range(HEADS):
                    nc.vector.tensor_scalar(
                        at[:, k * SHARD : (k + 1) * SHARD],
                        es8_b4[:, k * SHARD : (k + 1) * SHARD],
                        et82[:, k : k + 1],
                        et82[:, HEADS + k : HEADS + k + 1],
                        AluOpType.mult,
                        AluOpType.max,
                    )
                mview = (
                    msb[jt // 4][:, (jt % 4) * SHARD : (jt % 4 + 1) * SHARD]
                    .rearrange("p (one i) -> p one i", one=1)
                    .broadcast_to((128, HEADS, SHARD))
                )
                nc.vector.tensor_tensor(
                    at.rearrange("p (h i) -> p h i", h=HEADS),
                    at.rearrange("p (h i) -> p h i", h=HEADS),
                    mview,
                    AluOpType.mult,
                )
                for k in range(HEADS):
                    nc.tensor.matmul(
                        acc[k],
                        lhsT=h_sb[jt][:, k * HB : (k + 1) * HB],
                        rhs=at[:, k * SHARD : (k + 1) * SHARD],
                        start=(jt == 0),
                        stop=(jt == NT - 1),
                    )

            # ---------------- endgame: transpose, normalize, store
            # single out tile [128, (c, k*64+f)] -> one batched output DMA
            out_sb = ep.tile([128, (SHARD // 128) * HEADS * OUT_F], F32,
                             name="osb", tag="osb", bufs=1)
            osv = out_sb.rearrange("p (c f) -> p c f", c=SHARD // 128)
            for k in range(HEADS):
                num_sb = ep.tile([HB, SHARD], F32, name="num_sb", tag="num_sb")
                # alternate engines so the four PSUM->SBUF copies overlap
                if k % 2 == 0:
                    nc.vector.tensor_copy(num_sb, acc[k])
                else:
                    nc.scalar.copy(num_sb, acc[k])
                for c in range(SHARD // 128):
                    # alternate PSUM pools so 4 transposes stay in flight
                    pool = psp if (k * 4 + c) % 2 == 0 else php
                    tp = pool.tile([128, HB], F32, name="tp",
                                   tag="pstmp" if pool is psp else "ph")
                    nc.tensor.transpose(
                        tp, num_sb[:, c * 128 : (c + 1) * 128], ident[:HB, :HB]
                    )
                    rec = ep.tile([128, 1], F32, name="rec", tag="rec", bufs=4)
                    nc.vector.reciprocal(rec, tp[:, OUT_F : OUT_F + 1])
                    nc.scalar.activation(
                        osv[:, c, k * OUT_F : (k + 1) * OUT_F],
                        tp[:, :OUT_F],
                        act.Copy,
                        scale=rec,
                    )
            nc.sync.dma_start(
                out_d.rearrange("(c p) f -> p c f", p=128), osv
            )

    nc.finalize()
    _PROGRAM = nc
    return nc


# ------------------------------------------------------------------- driver
LAST_RESULT = None


def kernel(x, adj, W, a):
    global LAST_RESULT
    x = np.asarray(x, dtype=np.float32)
    adj = np.asarray(adj)
    W = np.asarray(W, dtype=np.float32)
    a = np.asarray(a, dtype=np.float32)

    # ---- host-side layout prep (sharding + transposes, no math on the data
    # beyond folding the tiny attention vectors into W)
    a_src = a[:OUT_F, 0]
    a_dst = a[OUT_F:, 0]
    Wh = W.reshape(IN_F, HEADS, OUT_F)
    Ws = np.einsum("fhc,c->fh", Wh, a_src)       # [IN_F, HEADS]
    Wt = np.einsum("fhc,c->fh", Wh, a_dst)
    wext = np.ascontiguousarray(
        np.concatenate([W, Ws, Wt, ALPHA * Wt], axis=1)
    ).astype(ml_dtypes.bfloat16)                 # [512, 268]

    wrep = np.ascontiguousarray(np.repeat(Ws, 128, axis=1)).astype(
        ml_dtypes.bfloat16
    )                                            # [512, 512]
    xT = np.ascontiguousarray(x.T).astype(ml_dtypes.bfloat16)   # [512, 4096]
    m01 = (adj.T != 0).astype(ml_dtypes.bfloat16)               # [4096, 4096]

    in_maps = []
    for c in range(NCORES):
        sl = slice(c * SHARD, (c + 1) * SHARD)
        # pre-tile the mask: [b, p, q*i] with row (q, i) contiguous (4KB lines)
        mtb = np.ascontiguousarray(
            m01[:, sl].reshape(NT // 4, 4, 128, SHARD)
            .transpose(0, 2, 1, 3)
            .reshape(NT // 4, 128, 4 * SHARD)
        )
        in_maps.append(
            {
                "xT": xT,
                "xsT": np.ascontiguousarray(xT[:, sl]),
                "wext": wext,
                "wrep": wrep,
                "maskTb": mtb,
            }
        )

    nc = _build_program()
    res = bass_utils.run_bass_kernel_spmd(
        nc,
        in_maps,
        core_ids=list(range(NCORES)),
        trace=os.environ.get("GAT_TRACE", "0") == "1",
    )
    LAST_RESULT = res
    out = np.concatenate([r["out"] for r in res.results], axis=0)
    return out.astype(np.float32)
